# revision 15
# baseline (speedup 1.0000x reference)
"""Trainium2 Bass kernel for a dense Clebsch-Gordan tensor product + per-irrep Linear.

Reference computation (e3nn-style):
  x1: [N, 1152] = 128x0e + 128x1o + 128x2e   (mul=128, l=0,1,2)
  x2: [N, 9]    = 1x0e + 1x1o + 1x2e         (spherical harmonics)
  y[n, (g, v*d3+k)] = sum_{paths p in g} sum_{u,i,j} w_g[slot_p*128+u, v]/sqrt(mul_g)
                       * cg_p[i,j,k] * x1_{l1}[n,u,i] * x2_{l2}[n,j]

Strategy (data-parallel over N across 8 cores; per core n=1024):
  - host precomputes one scaled weight matrix W~[u,v] per CG nonzero (244, fp16),
    plus fp16 relayouts x1T[u; i_glob, n] and x2rep[128; j_glob, n]
  - DVE builds outer-product columns OP_{ij}[u; n] = x1T[:, i, :] * x2rep[:, j, :]
  - one fp16 matmul per CG nonzero accumulates y_psum[(g,k)][v; n] += W~.T @ OP_{ij}
  - PSUM drains to fp16 and DMAs out TRANSPOSED as y_t[v; (g,k), n]; the host
    reassembles y[n, 5120] fp32 (no on-device transposes at all)
"""

import sys
from math import factorial

import numpy as np

if "/opt/trn_rl_repo" not in sys.path:
    sys.path.insert(0, "/opt/trn_rl_repo")

MUL = 128
N_TOTAL = 8192
N_CORES = 8
NPC = N_TOTAL // N_CORES  # 1024 rows per core
IN1 = [(0, 1), (1, -1), (2, 1)]
IN2 = [(0, 1), (1, -1), (2, 1)]

# --------------------------------------------------------------- CG tables ---


def _f(n):
    return float(factorial(n))


def _su2_cg(j1, j2, j3):
    C = np.zeros((2 * j1 + 1, 2 * j2 + 1, 2 * j3 + 1))
    if not (abs(j1 - j2) <= j3 <= j1 + j2):
        return C
    pref0 = np.sqrt((2 * j3 + 1) * _f(j1 + j2 - j3) * _f(j1 - j2 + j3) * _f(-j1 + j2 + j3) / _f(j1 + j2 + j3 + 1))
    for m1 in range(-j1, j1 + 1):
        for m2 in range(-j2, j2 + 1):
            m3 = m1 + m2
            if abs(m3) > j3:
                continue
            pref = pref0 * np.sqrt(_f(j3 + m3) * _f(j3 - m3) * _f(j1 - m1) * _f(j1 + m1) * _f(j2 - m2) * _f(j2 + m2))
            s = 0.0
            for k in range(j1 + j2 - j3 + 1):
                a = [k, j1 + j2 - j3 - k, j1 - m1 - k, j2 + m2 - k, j3 - j2 + m1 + k, j3 - j1 - m2 + k]
                if min(a) < 0:
                    continue
                s += (-1.0) ** k / np.prod([_f(t) for t in a])
            C[j1 + m1, j2 + m2, j3 + m3] = pref * s
    return C


def _q(l):
    q = np.zeros((2 * l + 1, 2 * l + 1), dtype=np.complex128)
    for m in range(-l, 0):
        q[l + m, l + abs(m)] = 1 / np.sqrt(2)
        q[l + m, l - abs(m)] = -1j / np.sqrt(2)
    q[l, l] = 1.0
    for m in range(1, l + 1):
        q[l + m, l + abs(m)] = (-1) ** m / np.sqrt(2)
        q[l + m, l - abs(m)] = 1j * (-1) ** m / np.sqrt(2)
    return (-1j) ** l * q


def _real_cg(l1, l2, l3):
    C = _su2_cg(l1, l2, l3).astype(np.complex128)
    C = np.einsum("ij,kl,mn,ikn->jlm", _q(l1), _q(l2), np.conj(_q(l3).T), C)
    return np.real(C)


PATHS = []
for (l1, p1) in IN1:
    for (l2, p2) in IN2:
        for l3 in range(abs(l1 - l2), l1 + l2 + 1):
            PATHS.append((l1, p1, l2, p2, l3, p1 * p2))
CG = {(l1, l2, l3): _real_cg(l1, l2, l3).astype(np.float32) for (l1, _, l2, _, l3, _) in PATHS}
GROUPS = sorted({(l3, p3) for (_, _, _, _, l3, p3) in PATHS})


def _gname(l, p):
    return "w%d%s" % (l, "e" if p == 1 else "o")


L1_OFF = {0: 0, 1: 1, 2: 4}   # i_glob = L1_OFF[l1] + i
X1_OFF = {0: 0, 1: 128, 2: 512}  # x1 flat col offset of l1 block
L2_OFF = {0: 0, 1: 1, 2: 4}   # j_glob = L2_OFF[l2] + j

MULS = {g: 0 for g in GROUPS}
for (_, _, _, _, l3, p3) in PATHS:
    MULS[(l3, p3)] += MUL

GOFF = {}
_off = 0
for g in GROUPS:
    GOFF[g] = _off
    _off += MUL * (2 * g[0] + 1)
assert _off == 5120


def _build_pass_list():
    """gk_passes: per (g,k) in output order, list of (ij, path_idx, coef)."""
    gk_passes = []
    for g in GROUPS:
        d3 = 2 * g[0] + 1
        for k in range(d3):
            contribs = []
            for pi, (l1, p1, l2, p2, l3, p3) in enumerate(PATHS):
                if (l3, p3) != g:
                    continue
                C = CG[(l1, l2, l3)]
                for i in range(2 * l1 + 1):
                    for j in range(2 * l2 + 1):
                        c = float(C[i, j, k])
                        if abs(c) < 1e-8:
                            continue
                        contribs.append(((L1_OFF[l1] + i, L2_OFF[l2] + j), pi, c))
            assert contribs
            gk_passes.append((g, k, contribs))
    used = []
    seen = set()
    for (_, _, contribs) in gk_passes:
        for (ij, _, _) in contribs:
            if ij not in seen:
                seen.add(ij)
                used.append(ij)
    return gk_passes, used


GK_PASSES, USED_IJ = _build_pass_list()


def _reorder_passes(gk_passes):
    """Schedule order for the 40 (g,k) outputs.

    Put the (1o,*) outputs first: their contributions only touch OP tiles
    ig 0..3, so the PE can start before the DVE has built all 9 outer-product
    tiles of chunk 0.  Put a minimal-work gk last so the final
    drain+DMA tail after the last matmul is as short as possible.
    """
    def igs_needed(entry):
        return {ij[0] for (ij, _, _) in entry[2]}

    first = [e for e in gk_passes if e[0] == (1, -1)]
    rest = [e for e in gk_passes if e[0] != (1, -1)]
    # last: fewest contribs
    last = min(rest, key=lambda e: len(e[2]))
    rest.remove(last)
    return first + rest + [last]


GK_PASSES = _reorder_passes(GK_PASSES)

# slab index of each (g, k) in the device output y_t (scheduling order)
SLAB_OF = {(g, k): idx for idx, (g, k, _) in enumerate(GK_PASSES)}
N_GK = len(GK_PASSES)  # 40

# slot groups: first group = 3x(1o), then 4s, last = the small single gk
SLOT_GROUPS = [[0, 1, 2]] + [[3 + 4 * t + s for s in range(4)] for t in range(9)] + [[39]]
assert sorted(sum(SLOT_GROUPS, [])) == list(range(N_GK))

N_PASSES = sum(len(c) for (_, _, c) in GK_PASSES)  # 244

# path-major stacking of the scaled weights: the on-device weight stack is
# built as W_p (x) c_vec per path (19 broadcast multiplies on gpsimd) instead
# of DMAing 244 pre-scaled copies (8MB) from HBM.  PASS_POS maps each
# (gk-order) pass to its slot in the path-major stack.
PATH_PASS = {pi: [] for pi in range(len(PATHS))}   # path -> [(gidx, m, coef)]
for gidx, (_, _, contribs) in enumerate(GK_PASSES):
    for m, (_, pi, c) in enumerate(contribs):
        PATH_PASS[pi].append((gidx, m, c))
PATH_RANGE = {}   # path -> (start, end) in the path-major stack
PASS_POS = {}     # (gidx, m) -> stack index
PASS_COEF = np.zeros(N_PASSES, np.float32)
_pos = 0
for pi in range(len(PATHS)):
    r0 = _pos
    for (gidx, m, c) in PATH_PASS[pi]:
        PASS_POS[(gidx, m)] = _pos
        PASS_COEF[_pos] = c
        _pos += 1
    PATH_RANGE[pi] = (r0, _pos)
assert _pos == N_PASSES


def _host_prep(inputs):
    """Host-side layout prep: x1T, x2rep (fp16) and the scaled weight stack."""
    x1 = np.asarray(inputs["x1"], np.float32)
    x2 = np.asarray(inputs["x2"], np.float32)
    n = x1.shape[0]

    x1t = np.empty((128, 9, n), np.float16)
    for (l1, _) in IN1:
        d1 = 2 * l1 + 1
        blk = x1[:, X1_OFF[l1]:X1_OFF[l1] + MUL * d1].reshape(n, MUL, d1)
        for i in range(d1):
            x1t[:, L1_OFF[l1] + i, :] = blk[:, :, i].astype(np.float16).T

    x2t = x2.astype(np.float16).T  # [9, n]
    x2rep = np.ascontiguousarray(np.broadcast_to(x2t[None, :, :], (128, 9, n)))

    # per-path weight slices (with e3nn path normalization)
    W = {g: np.asarray(inputs[_gname(*g)], np.float32) for g in GROUPS}
    slot = {g: 0 for g in GROUPS}
    path_w = []
    for (l1, p1, l2, p2, l3, p3) in PATHS:
        g = (l3, p3)
        s = slot[g]
        slot[g] += 1
        path_w.append(W[g][s * MUL:(s + 1) * MUL, :] / np.sqrt(np.float32(MULS[g])))

    wp = np.empty((MUL, len(PATHS), MUL), np.float16)   # [u, path, v]
    for pi in range(len(PATHS)):
        wp[:, pi, :] = path_w[pi].astype(np.float16)
    cvec = np.ascontiguousarray(
        np.broadcast_to(PASS_COEF.astype(np.float16)[None, :, None],
                        (128, N_PASSES, 1)))
    return x1t, x2rep, wp, cvec


def _host_assemble(yt_cores):
    """yt per core: [128(v), N_GK, npc] -> y [N, 5120] fp32."""
    n_cores = len(yt_cores)
    npc = yt_cores[0].shape[2]
    y = np.empty((n_cores * npc, 5120), np.float32)
    for ci, yt in enumerate(yt_cores):
        rows = slice(ci * npc, (ci + 1) * npc)
        for g in GROUPS:
            d3 = 2 * g[0] + 1
            slabs = [SLAB_OF[(g, k)] for k in range(d3)]
            blk = yt[:, slabs, :]                   # [v, k, n]
            y[rows, GOFF[g]:GOFF[g] + MUL * d3] = (
                blk.transpose(2, 0, 1).reshape(npc, MUL * d3)
            )
    return y


# --------------------------------------------------------------- bass build ---

_CACHE = {}


def _build(n_per_core=NPC, chunks="128,256,256,256,128", op_bufs=18,
           yacc_bufs=4, ystg_bufs=6, act_share=0):
    """Build the per-core Bass/Tile program (v3: transposed output).

    Layouts:
      x1t  [u; i_glob(9), n]   fp16 (host-prepped); per-chunk SBUF tiles
      x2r  [128; j_glob(9), n] fp16 (host-prepped broadcast); per-chunk tiles
      wt   [u; pass(244), v]   fp16 scaled weights in scheduling order,
                               split into one SBUF tile per slot group
      y_t  [128(v), N_GK(40), n] fp16 output (host assembles y[n, 5120] fp32)

    Per n-chunk: DVE builds outer-product tiles OP_(i,*)[u; j, n]; one
    PSUM-accumulating fp16 matmul per CG nonzero into per-slot-group
    accumulator tiles [v; slot, n]; Act drains slots to fp16 staging; DMA
    writes the staging tile straight into y_t's (gk, chunk) slab.
    No PE transposes.
    """
    import concourse.bass as bass
    import concourse.mybir as mybir
    import concourse.tile as tile

    dt = mybir.dt
    NCROWS = n_per_core
    if isinstance(chunks, str):
        chunks = [int(x) for x in chunks.split(",")]
    assert sum(chunks) == NCROWS
    CH = len(chunks)
    coff = [sum(chunks[:c]) for c in range(CH)]

    flat = GK_PASSES
    n_gk = len(flat)
    groups = SLOT_GROUPS
    # every slot group must be a contiguous run of gk indices
    for grp in groups:
        assert grp == list(range(grp[0], grp[0] + len(grp)))

    # pass-index ranges per slot group (wt is gk-major so these are contiguous)
    grp_pass_range = []
    pc = 0
    for grp in groups:
        p0 = pc
        for gidx in grp:
            pc += len(flat[gidx][2])
        grp_pass_range.append((p0, pc))
    assert pc == N_PASSES

    nc = bass.Bass()

    x1t_d = nc.dram_tensor("x1t", [128, 9, NCROWS], dt.float16, kind="ExternalInput")
    x2r_d = nc.dram_tensor("x2r", [128, 9, NCROWS], dt.float16, kind="ExternalInput")
    wp_d = nc.dram_tensor("wp", [MUL, len(PATHS), MUL], dt.float16, kind="ExternalInput")
    cv_d = nc.dram_tensor("cv", [128, N_PASSES, 1], dt.float16, kind="ExternalInput")
    yt_d = nc.dram_tensor("yt", [128, n_gk, NCROWS], dt.float16, kind="ExternalOutput")

    with tile.TileContext(nc) as tc:
        with (
            tc.tile_pool(name="const", bufs=1) as constp,
            tc.tile_pool(name="op", bufs=op_bufs) as opp,
            tc.tile_pool(name="ystg", bufs=ystg_bufs) as ystgp,
            tc.tile_pool(name="yacc", bufs=yacc_bufs, space="PSUM") as yaccp,
        ):
            x1c = [constp.tile([128, 9, chunks[c]], dt.float16, name=f"x1c{c}")
                   for c in range(CH)]
            x2c = [constp.tile([128, 9, chunks[c]], dt.float16, name=f"x2c{c}")
                   for c in range(CH)]
            wp_sb = constp.tile([128, len(PATHS), 128], dt.float16, name="wp")
            cv_sb = constp.tile([128, N_PASSES, 1], dt.float16, name="cv")
            wt_sb = constp.tile([128, N_PASSES, 128], dt.float16, name="wt_stack")

            # input DMAs on the SP queue (total only ~5.4MB now)
            nc.sync.dma_start(x1c[0][:], x1t_d[:, :, 0:chunks[0]])
            nc.sync.dma_start(x2c[0][:], x2r_d[:, :, 0:chunks[0]])
            nc.sync.dma_start(wp_sb[:], wp_d[:])
            nc.sync.dma_start(cv_sb[:], cv_d[:])
            for c in range(1, CH):
                n0 = coff[c]
                nc.sync.dma_start(x1c[c][:], x1t_d[:, :, n0:n0 + chunks[c]])
                nc.sync.dma_start(x2c[c][:], x2r_d[:, :, n0:n0 + chunks[c]])

            # build the 244-pass scaled weight stack on the otherwise-idle
            # Pool engine: one broadcast multiply per path, ordered by first
            # use so early matmuls aren't blocked by late paths
            path_first_use = {}
            order = 0
            for grp in groups:
                for gidx in grp:
                    for m, (_, pi, _) in enumerate(GK_PASSES[gidx][2]):
                        path_first_use.setdefault(pi, order)
                        order += 1
            for pi in sorted(range(len(PATHS)), key=lambda p: path_first_use[p]):
                r0, r1 = PATH_RANGE[pi]
                w_b = wp_sb[:, pi, :].unsqueeze(1).broadcast_to([128, r1 - r0, 128])
                c_b = cv_sb[:, r0:r1, :].broadcast_to([128, r1 - r0, 128])
                nc.gpsimd.tensor_mul(wt_sb[:, r0:r1, :], w_b, c_b)

            for c in range(CH):
                n0 = coff[c]
                ncols = chunks[c]

                op_tiles = {}
                for ig in range(9):
                    t_ = opp.tile([128, 9, ncols], dt.float16, tag="op",
                                  name=f"op_c{c}_i{ig}")
                    a_b = x1c[c][:, ig, :].unsqueeze(1)
                    a_b = a_b.broadcast_to([128, 9, ncols])
                    nc.vector.tensor_mul(t_[:], a_b, x2c[c][:])
                    for jg in range(9):
                        op_tiles[(ig, jg)] = t_[:, jg, :]

                for t_idx, grp in enumerate(groups):
                    gsz = len(grp)
                    acc = yaccp.tile([128, gsz, ncols], dt.float32,
                                     tag="yacc", name=f"acc_c{c}_t{t_idx}")
                    ystg = ystgp.tile([128, gsz, ncols], dt.float16,
                                      tag="ystg", name=f"ystg_c{c}_t{t_idx}")
                    for s, gidx in enumerate(grp):
                        (g, k, contribs) = flat[gidx]
                        nmm = len(contribs)
                        for m, (ij, _, _) in enumerate(contribs):
                            nc.tensor.matmul(
                                acc[:, s, :],
                                wt_sb[:, PASS_POS[(gidx, m)], :],
                                op_tiles[ij],
                                start=(m == 0),
                                stop=(m == nmm - 1),
                            )
                    # alternate the PSUM->fp16 drains between Act and DVE so
                    # neither engine's drain chain lags the PE's PSUM ring
                    # (gpsimd cannot access PSUM)
                    if (c * len(groups) + t_idx) % 2 == 0:
                        nc.scalar.copy(ystg[:], acc[:])
                    else:
                        nc.vector.tensor_copy(ystg[:], acc[:])
                    s0 = grp[0]
                    nc.scalar.dma_start(
                        yt_d[:, s0:s0 + gsz, n0:n0 + ncols], ystg[:]
                    )

    return nc


def _hoist_waits(nc, max_waits=1):
    """Walrus in this toolchain rejects instructions with more than one
    sync-wait command; hoist extras onto single-wait NOP/Drain carriers that
    precede the instruction on the same engine."""
    import concourse.mybir as mybir

    n_hoisted = 0
    for bb in nc.main_func.blocks:
        new_list = []
        for ins in bb.instructions:
            si = ins.sync_info
            if si is not None and si.on_wait and len(si.on_wait) > max_waits:
                waits = list(si.on_wait)
                keep, hoist = waits[:max_waits], waits[max_waits:]
                for w in hoist:
                    carrier = mybir.InstDrain(
                        name=nc.get_next_instruction_name(),
                        ins=[], outs=[], bass_is_fusable=False)
                    carrier.engine = ins.engine
                    carrier.sync_info = mybir.SyncInfo(on_wait=[w], on_update=[])
                    new_list.append(carrier)
                    n_hoisted += 1
                ins.sync_info = mybir.SyncInfo(on_wait=keep, on_update=list(si.on_update))
            new_list.append(ins)
        bb.instructions[:] = new_list
    return n_hoisted


def _get_program(**kw):
    key = tuple(sorted(kw.items()))
    if key not in _CACHE:
        nc = _build(**kw)
        _hoist_waits(nc)
        _CACHE[key] = nc
    return _CACHE[key]


def _run(inputs, trace=False, **build_kw):
    from concourse import bass_utils

    nc = _get_program(**build_kw)
    x1t, x2rep, wp, cvec = _host_prep(inputs)

    in_maps = []
    for core in range(N_CORES):
        sl = slice(core * NPC, (core + 1) * NPC)
        in_maps.append({
            "x1t": np.ascontiguousarray(x1t[:, :, sl]),
            "x2r": np.ascontiguousarray(x2rep[:, :, sl]),
            "wp": wp,
            "cv": cvec,
        })

    res = bass_utils.run_bass_kernel_spmd(
        nc, in_maps, core_ids=list(range(N_CORES)), trace=trace,
    )
    y = _host_assemble([r["yt"] for r in res.results])
    return y, res


def kernel(**inputs):
    y, _ = _run(inputs)
    return y


def _make_sharded_fn(nc):
    """Mimic bass2jax.run_bass_via_pjrt's multi-core path, returning
    (sharded_fn, in_names, out_names, out_avals, mesh, n_params)."""
    import jax
    from jax.sharding import Mesh, PartitionSpec
    from jax.experimental.shard_map import shard_map
    from concourse import bass2jax, mybir
    import numpy as _np

    bass2jax.install_neuronx_cc_hook()
    partition_name = nc.partition_id_tensor.name if nc.partition_id_tensor else None
    in_names, out_names, out_avals = [], [], []
    for alloc in nc.m.functions[0].allocations:
        if not isinstance(alloc, mybir.MemoryLocationSet):
            continue
        name = alloc.memorylocations[0].name
        if alloc.kind == "ExternalInput":
            if name != partition_name:
                in_names.append(name)
        elif alloc.kind == "ExternalOutput":
            out_names.append(name)
            out_avals.append(jax.core.ShapedArray(tuple(alloc.tensor_shape), mybir.dt.np(alloc.dtype)))
    n_params = len(in_names)
    all_in_names = list(in_names) + list(out_names)
    if partition_name is not None:
        all_in_names.append(partition_name)
    donate = tuple(range(n_params, n_params + len(out_avals)))

    def _body(*args):
        operands = list(args)
        if partition_name is not None:
            operands.append(bass2jax.partition_id_tensor())
        outs = bass2jax._bass_exec_p.bind(
            *operands,
            out_avals=tuple(out_avals),
            in_names=tuple(all_in_names),
            out_names=tuple(out_names),
            lowering_input_output_aliases=(),
            sim_require_finite=True,
            sim_require_nnan=True,
            nc=nc,
        )
        return tuple(outs)

    devices = jax.devices()[:N_CORES]
    mesh = Mesh(_np.asarray(devices), ("core",))
    in_specs = (PartitionSpec("core"),) * (n_params + len(out_avals))
    out_specs = (PartitionSpec("core"),) * len(out_names)
    sharded = jax.jit(
        shard_map(_body, mesh=mesh, in_specs=in_specs, out_specs=out_specs, check_rep=False),
        donate_argnums=donate,
        keep_unused=True,
    )
    return sharded, in_names, out_names, out_avals, mesh, n_params


def bench(inputs, iters=30, warmup=3, **build_kw):
    """Time repeated on-device executions (inputs device-resident).

    Returns (per_iter_ns, y) where per_iter_ns is the best average over
    the timed iterations.
    """
    import time as _time
    import jax
    from jax.sharding import NamedSharding, PartitionSpec

    nc = _get_program(**build_kw)
    x1t, x2rep, wp, cvec = _host_prep(inputs)
    per_core = {
        "x1t": [np.ascontiguousarray(x1t[:, :, slice(c * NPC, (c + 1) * NPC)]) for c in range(N_CORES)],
        "x2r": [np.ascontiguousarray(x2rep[:, :, slice(c * NPC, (c + 1) * NPC)]) for c in range(N_CORES)],
        "wp": [wp] * N_CORES,
        "cv": [cvec] * N_CORES,
    }
    sharded, in_names, out_names, out_avals, mesh, n_params = _make_sharded_fn(nc)
    sh = NamedSharding(mesh, PartitionSpec("core"))
    dev_in = [
        jax.device_put(np.concatenate(per_core[name], axis=0), sh) for name in in_names
    ]

    def zeros():
        return [
            jax.device_put(np.zeros((N_CORES * a.shape[0], *a.shape[1:]), a.dtype), sh)
            for a in out_avals
        ]

    outs = None
    for _ in range(warmup):
        outs = sharded(*dev_in, *zeros())
        jax.block_until_ready(outs)

    # pre-stage zero buffers outside the timed region
    zs = [zeros() for _ in range(iters)]
    jax.block_until_ready(zs)
    t0 = _time.perf_counter()
    res = [sharded(*dev_in, *z) for z in zs]
    jax.block_until_ready(res)
    dt = (_time.perf_counter() - t0) / iters

    y_cat = np.asarray(res[-1][out_names.index("yt")])
    yt_cores = [y_cat[c * 128:(c + 1) * 128] for c in range(N_CORES)]
    y = _host_assemble(yt_cores)
    return dt * 1e9, y


if __name__ == "__main__":
    print("passes:", N_PASSES, "used_ij:", len(USED_IJ))


# revision 17
# speedup vs baseline: 1.2671x; 1.2671x over previous
"""Trainium2 Bass kernel for a dense Clebsch-Gordan tensor product + per-irrep Linear.

Reference computation (e3nn-style):
  x1: [N, 1152] = 128x0e + 128x1o + 128x2e   (mul=128, l=0,1,2)
  x2: [N, 9]    = 1x0e + 1x1o + 1x2e         (spherical harmonics)
  y[n, (g, v*d3+k)] = sum_{paths p in g} sum_{u,i,j} w_g[slot_p*128+u, v]/sqrt(mul_g)
                       * cg_p[i,j,k] * x1_{l1}[n,u,i] * x2_{l2}[n,j]

Strategy (data-parallel over N across 8 cores; per core n=1024):
  - host precomputes one scaled weight matrix W~[u,v] per CG nonzero (244, fp16),
    plus fp16 relayouts x1T[u; i_glob, n] and x2rep[128; j_glob, n]
  - DVE builds outer-product columns OP_{ij}[u; n] = x1T[:, i, :] * x2rep[:, j, :]
  - one fp16 matmul per CG nonzero accumulates y_psum[(g,k)][v; n] += W~.T @ OP_{ij}
  - PSUM drains to fp16 and DMAs out TRANSPOSED as y_t[v; (g,k), n]; the host
    reassembles y[n, 5120] fp32 (no on-device transposes at all)
"""

import sys
from math import factorial

import numpy as np

if "/opt/trn_rl_repo" not in sys.path:
    sys.path.insert(0, "/opt/trn_rl_repo")

MUL = 128
N_TOTAL = 8192
N_CORES = 8
NPC = N_TOTAL // N_CORES  # 1024 rows per core
IN1 = [(0, 1), (1, -1), (2, 1)]
IN2 = [(0, 1), (1, -1), (2, 1)]

# --------------------------------------------------------------- CG tables ---


def _f(n):
    return float(factorial(n))


def _su2_cg(j1, j2, j3):
    C = np.zeros((2 * j1 + 1, 2 * j2 + 1, 2 * j3 + 1))
    if not (abs(j1 - j2) <= j3 <= j1 + j2):
        return C
    pref0 = np.sqrt((2 * j3 + 1) * _f(j1 + j2 - j3) * _f(j1 - j2 + j3) * _f(-j1 + j2 + j3) / _f(j1 + j2 + j3 + 1))
    for m1 in range(-j1, j1 + 1):
        for m2 in range(-j2, j2 + 1):
            m3 = m1 + m2
            if abs(m3) > j3:
                continue
            pref = pref0 * np.sqrt(_f(j3 + m3) * _f(j3 - m3) * _f(j1 - m1) * _f(j1 + m1) * _f(j2 - m2) * _f(j2 + m2))
            s = 0.0
            for k in range(j1 + j2 - j3 + 1):
                a = [k, j1 + j2 - j3 - k, j1 - m1 - k, j2 + m2 - k, j3 - j2 + m1 + k, j3 - j1 - m2 + k]
                if min(a) < 0:
                    continue
                s += (-1.0) ** k / np.prod([_f(t) for t in a])
            C[j1 + m1, j2 + m2, j3 + m3] = pref * s
    return C


def _q(l):
    q = np.zeros((2 * l + 1, 2 * l + 1), dtype=np.complex128)
    for m in range(-l, 0):
        q[l + m, l + abs(m)] = 1 / np.sqrt(2)
        q[l + m, l - abs(m)] = -1j / np.sqrt(2)
    q[l, l] = 1.0
    for m in range(1, l + 1):
        q[l + m, l + abs(m)] = (-1) ** m / np.sqrt(2)
        q[l + m, l - abs(m)] = 1j * (-1) ** m / np.sqrt(2)
    return (-1j) ** l * q


def _real_cg(l1, l2, l3):
    C = _su2_cg(l1, l2, l3).astype(np.complex128)
    C = np.einsum("ij,kl,mn,ikn->jlm", _q(l1), _q(l2), np.conj(_q(l3).T), C)
    return np.real(C)


PATHS = []
for (l1, p1) in IN1:
    for (l2, p2) in IN2:
        for l3 in range(abs(l1 - l2), l1 + l2 + 1):
            PATHS.append((l1, p1, l2, p2, l3, p1 * p2))
CG = {(l1, l2, l3): _real_cg(l1, l2, l3).astype(np.float32) for (l1, _, l2, _, l3, _) in PATHS}
GROUPS = sorted({(l3, p3) for (_, _, _, _, l3, p3) in PATHS})


def _gname(l, p):
    return "w%d%s" % (l, "e" if p == 1 else "o")


L1_OFF = {0: 0, 1: 1, 2: 4}   # i_glob = L1_OFF[l1] + i
X1_OFF = {0: 0, 1: 128, 2: 512}  # x1 flat col offset of l1 block
L2_OFF = {0: 0, 1: 1, 2: 4}   # j_glob = L2_OFF[l2] + j

MULS = {g: 0 for g in GROUPS}
for (_, _, _, _, l3, p3) in PATHS:
    MULS[(l3, p3)] += MUL

GOFF = {}
_off = 0
for g in GROUPS:
    GOFF[g] = _off
    _off += MUL * (2 * g[0] + 1)
assert _off == 5120


def _build_pass_list():
    """gk_passes: per (g,k) in output order, list of (ij, path_idx, coef)."""
    gk_passes = []
    for g in GROUPS:
        d3 = 2 * g[0] + 1
        for k in range(d3):
            contribs = []
            for pi, (l1, p1, l2, p2, l3, p3) in enumerate(PATHS):
                if (l3, p3) != g:
                    continue
                C = CG[(l1, l2, l3)]
                for i in range(2 * l1 + 1):
                    for j in range(2 * l2 + 1):
                        c = float(C[i, j, k])
                        if abs(c) < 1e-8:
                            continue
                        contribs.append(((L1_OFF[l1] + i, L2_OFF[l2] + j), pi, c))
            assert contribs
            gk_passes.append((g, k, contribs))
    used = []
    seen = set()
    for (_, _, contribs) in gk_passes:
        for (ij, _, _) in contribs:
            if ij not in seen:
                seen.add(ij)
                used.append(ij)
    return gk_passes, used


GK_PASSES, USED_IJ = _build_pass_list()


def _reorder_passes(gk_passes):
    """Schedule order for the 40 (g,k) outputs.

    Put the (1o,*) outputs first: their contributions only touch OP tiles
    ig 0..3, so the PE can start before the DVE has built all 9 outer-product
    tiles of chunk 0.  Put a minimal-work gk last so the final
    drain+DMA tail after the last matmul is as short as possible.
    """
    def igs_needed(entry):
        return {ij[0] for (ij, _, _) in entry[2]}

    first = [e for e in gk_passes if e[0] == (1, -1)]
    rest = [e for e in gk_passes if e[0] != (1, -1)]
    # last: fewest contribs
    last = min(rest, key=lambda e: len(e[2]))
    rest.remove(last)
    return first + rest + [last]


GK_PASSES = _reorder_passes(GK_PASSES)

# slab index of each (g, k) in the device output y_t (scheduling order)
SLAB_OF = {(g, k): idx for idx, (g, k, _) in enumerate(GK_PASSES)}
N_GK = len(GK_PASSES)  # 40

# slot groups: first group = 3x(1o), then 4s, last = the small single gk
SLOT_GROUPS = [[0, 1, 2]] + [[3 + 4 * t + s for s in range(4)] for t in range(9)] + [[39]]
assert sorted(sum(SLOT_GROUPS, [])) == list(range(N_GK))

N_PASSES = sum(len(c) for (_, _, c) in GK_PASSES)  # 244

# Many passes share the same (path, coefficient): only 89 of the 244 scaled
# weight matrices are distinct.  The device weight stack holds one slab per
# unique (path, coef), ordered by first use in the schedule; PASS_POS maps
# each (gk-order, contrib) pass to its slab.
_uniq = {}
PASS_POS = {}
WT_SLABS = []   # [(path_idx, coef)] in first-use order
for gidx, (_, _, contribs) in enumerate(GK_PASSES):
    for m, (_, pi, c) in enumerate(contribs):
        key = (pi, round(c, 7))
        if key not in _uniq:
            _uniq[key] = len(WT_SLABS)
            WT_SLABS.append((pi, c))
        PASS_POS[(gidx, m)] = _uniq[key]
N_SLABS = len(WT_SLABS)  # 89
# last slab index needed by each slot group (for JIT weight DMA split)
GRP_MAX_SLAB = []
for grp in SLOT_GROUPS:
    mx = 0
    for gidx in grp:
        for m in range(len(GK_PASSES[gidx][2])):
            mx = max(mx, PASS_POS[(gidx, m)])
    GRP_MAX_SLAB.append(mx)


def _host_prep(inputs):
    """Host-side layout prep: x1T, x2rep (fp16) and the scaled weight stack."""
    x1 = np.asarray(inputs["x1"], np.float32)
    x2 = np.asarray(inputs["x2"], np.float32)
    n = x1.shape[0]

    x1t = np.empty((128, 9, n), np.float16)
    for (l1, _) in IN1:
        d1 = 2 * l1 + 1
        blk = x1[:, X1_OFF[l1]:X1_OFF[l1] + MUL * d1].reshape(n, MUL, d1)
        for i in range(d1):
            x1t[:, L1_OFF[l1] + i, :] = blk[:, :, i].astype(np.float16).T

    x2t = x2.astype(np.float16).T  # [9, n]
    x2rep = np.ascontiguousarray(np.broadcast_to(x2t[None, :, :], (128, 9, n)))

    # per-path weight slices (with e3nn path normalization)
    W = {g: np.asarray(inputs[_gname(*g)], np.float32) for g in GROUPS}
    slot = {g: 0 for g in GROUPS}
    path_w = []
    for (l1, p1, l2, p2, l3, p3) in PATHS:
        g = (l3, p3)
        s = slot[g]
        slot[g] += 1
        path_w.append(W[g][s * MUL:(s + 1) * MUL, :] / np.sqrt(np.float32(MULS[g])))

    wt = np.empty((MUL, N_SLABS, MUL), np.float16)   # [u, slab, v]
    for si, (pi, c) in enumerate(WT_SLABS):
        wt[:, si, :] = (path_w[pi] * np.float32(c)).astype(np.float16)
    return x1t, x2rep, wt


def _host_assemble(yt_cores):
    """yt per core: [128(v), N_GK, npc] -> y [N, 5120] fp32."""
    n_cores = len(yt_cores)
    npc = yt_cores[0].shape[2]
    y = np.empty((n_cores * npc, 5120), np.float32)
    for ci, yt in enumerate(yt_cores):
        rows = slice(ci * npc, (ci + 1) * npc)
        for g in GROUPS:
            d3 = 2 * g[0] + 1
            slabs = [SLAB_OF[(g, k)] for k in range(d3)]
            blk = yt[:, slabs, :]                   # [v, k, n]
            y[rows, GOFF[g]:GOFF[g] + MUL * d3] = (
                blk.transpose(2, 0, 1).reshape(npc, MUL * d3)
            )
    return y


# --------------------------------------------------------------- bass build ---

_CACHE = {}


def _build(n_per_core=NPC, chunks="128,256,256,256,128", op_bufs=18,
           yacc_bufs=4, ystg_bufs=6, act_share=0):
    """Build the per-core Bass/Tile program (v3: transposed output).

    Layouts:
      x1t  [u; i_glob(9), n]   fp16 (host-prepped); per-chunk SBUF tiles
      x2r  [128; j_glob(9), n] fp16 (host-prepped broadcast); per-chunk tiles
      wt   [u; pass(244), v]   fp16 scaled weights in scheduling order,
                               split into one SBUF tile per slot group
      y_t  [128(v), N_GK(40), n] fp16 output (host assembles y[n, 5120] fp32)

    Per n-chunk: DVE builds outer-product tiles OP_(i,*)[u; j, n]; one
    PSUM-accumulating fp16 matmul per CG nonzero into per-slot-group
    accumulator tiles [v; slot, n]; Act drains slots to fp16 staging; DMA
    writes the staging tile straight into y_t's (gk, chunk) slab.
    No PE transposes.
    """
    import concourse.bass as bass
    import concourse.mybir as mybir
    import concourse.tile as tile

    dt = mybir.dt
    NCROWS = n_per_core
    if isinstance(chunks, str):
        chunks = [int(x) for x in chunks.split(",")]
    assert sum(chunks) == NCROWS
    CH = len(chunks)
    coff = [sum(chunks[:c]) for c in range(CH)]

    flat = GK_PASSES
    n_gk = len(flat)
    groups = SLOT_GROUPS
    # every slot group must be a contiguous run of gk indices
    for grp in groups:
        assert grp == list(range(grp[0], grp[0] + len(grp)))

    # pass-index ranges per slot group (wt is gk-major so these are contiguous)
    grp_pass_range = []
    pc = 0
    for grp in groups:
        p0 = pc
        for gidx in grp:
            pc += len(flat[gidx][2])
        grp_pass_range.append((p0, pc))
    assert pc == N_PASSES

    nc = bass.Bass()

    x1t_d = nc.dram_tensor("x1t", [128, 9, NCROWS], dt.float16, kind="ExternalInput")
    x2r_d = nc.dram_tensor("x2r", [128, 9, NCROWS], dt.float16, kind="ExternalInput")
    wt_d = nc.dram_tensor("wt", [MUL, N_SLABS, MUL], dt.float16, kind="ExternalInput")
    yt_d = nc.dram_tensor("yt", [128, n_gk, NCROWS], dt.float16, kind="ExternalOutput")

    with tile.TileContext(nc) as tc:
        with (
            tc.tile_pool(name="const", bufs=1) as constp,
            tc.tile_pool(name="op", bufs=op_bufs) as opp,
            tc.tile_pool(name="ystg", bufs=ystg_bufs) as ystgp,
            tc.tile_pool(name="yacc", bufs=yacc_bufs, space="PSUM") as yaccp,
        ):
            x1c = [constp.tile([128, 9, chunks[c]], dt.float16, name=f"x1c{c}")
                   for c in range(CH)]
            x2c = [constp.tile([128, 9, chunks[c]], dt.float16, name=f"x2c{c}")
                   for c in range(CH)]
            wt_sb = constp.tile([128, N_SLABS, 128], dt.float16, name="wt_stack")

            # chunk-0 x + the weight slabs go just-in-time on the SP queue;
            # the later chunks' x tiles go on the Act queue (idle early, its
            # output DMAs only start once compute is underway).  Weight DMA
            # is split at the slab boundaries the first chunk's slot groups
            # need, in first-use order.
            nc.sync.dma_start(x1c[0][:], x1t_d[:, :, 0:chunks[0]])
            nc.sync.dma_start(x2c[0][:], x2r_d[:, :, 0:chunks[0]])
            for c in range(1, CH):
                n0 = coff[c]
                nc.scalar.dma_start(x1c[c][:], x1t_d[:, :, n0:n0 + chunks[c]])
                nc.scalar.dma_start(x2c[c][:], x2r_d[:, :, n0:n0 + chunks[c]])
            s_prev = 0
            for t in range(len(groups)):
                s_end = max(GRP_MAX_SLAB[t] + 1, s_prev)
                if s_end > s_prev:
                    nc.sync.dma_start(wt_sb[:, s_prev:s_end, :],
                                      wt_d[:, s_prev:s_end, :])
                s_prev = s_end
            if s_prev < N_SLABS:
                nc.sync.dma_start(wt_sb[:, s_prev:, :], wt_d[:, s_prev:, :])

            for c in range(CH):
                n0 = coff[c]
                ncols = chunks[c]

                op_tiles = {}
                for ig in range(9):
                    t_ = opp.tile([128, 9, ncols], dt.float16, tag="op",
                                  name=f"op_c{c}_i{ig}")
                    a_b = x1c[c][:, ig, :].unsqueeze(1)
                    a_b = a_b.broadcast_to([128, 9, ncols])
                    nc.vector.tensor_mul(t_[:], a_b, x2c[c][:])
                    for jg in range(9):
                        op_tiles[(ig, jg)] = t_[:, jg, :]

                for t_idx, grp in enumerate(groups):
                    gsz = len(grp)
                    acc = yaccp.tile([128, gsz, ncols], dt.float32,
                                     tag="yacc", name=f"acc_c{c}_t{t_idx}")
                    ystg = ystgp.tile([128, gsz, ncols], dt.float16,
                                      tag="ystg", name=f"ystg_c{c}_t{t_idx}")
                    for s, gidx in enumerate(grp):
                        (g, k, contribs) = flat[gidx]
                        nmm = len(contribs)
                        for m, (ij, _, _) in enumerate(contribs):
                            nc.tensor.matmul(
                                acc[:, s, :],
                                wt_sb[:, PASS_POS[(gidx, m)], :],
                                op_tiles[ij],
                                start=(m == 0),
                                stop=(m == nmm - 1),
                            )
                    # alternate the PSUM->fp16 drains between Act and DVE so
                    # neither engine's drain chain lags the PE's PSUM ring
                    # (gpsimd cannot access PSUM)
                    if (c * len(groups) + t_idx) % 2 == 0:
                        nc.scalar.copy(ystg[:], acc[:])
                    else:
                        nc.vector.tensor_copy(ystg[:], acc[:])
                    s0 = grp[0]
                    nc.scalar.dma_start(
                        yt_d[:, s0:s0 + gsz, n0:n0 + ncols], ystg[:]
                    )

    return nc


def _hoist_waits(nc, max_waits=1):
    """Walrus in this toolchain rejects instructions with more than one
    sync-wait command; hoist extras onto single-wait NOP/Drain carriers that
    precede the instruction on the same engine."""
    import concourse.mybir as mybir

    n_hoisted = 0
    for bb in nc.main_func.blocks:
        new_list = []
        for ins in bb.instructions:
            si = ins.sync_info
            if si is not None and si.on_wait and len(si.on_wait) > max_waits:
                waits = list(si.on_wait)
                keep, hoist = waits[:max_waits], waits[max_waits:]
                for w in hoist:
                    carrier = mybir.InstDrain(
                        name=nc.get_next_instruction_name(),
                        ins=[], outs=[], bass_is_fusable=False)
                    carrier.engine = ins.engine
                    carrier.sync_info = mybir.SyncInfo(on_wait=[w], on_update=[])
                    new_list.append(carrier)
                    n_hoisted += 1
                ins.sync_info = mybir.SyncInfo(on_wait=keep, on_update=list(si.on_update))
            new_list.append(ins)
        bb.instructions[:] = new_list
    return n_hoisted


def _get_program(**kw):
    key = tuple(sorted(kw.items()))
    if key not in _CACHE:
        nc = _build(**kw)
        _hoist_waits(nc)
        _CACHE[key] = nc
    return _CACHE[key]


def _run(inputs, trace=False, **build_kw):
    from concourse import bass_utils

    nc = _get_program(**build_kw)
    x1t, x2rep, wt = _host_prep(inputs)

    in_maps = []
    for core in range(N_CORES):
        sl = slice(core * NPC, (core + 1) * NPC)
        in_maps.append({
            "x1t": np.ascontiguousarray(x1t[:, :, sl]),
            "x2r": np.ascontiguousarray(x2rep[:, :, sl]),
            "wt": wt,
        })

    res = bass_utils.run_bass_kernel_spmd(
        nc, in_maps, core_ids=list(range(N_CORES)), trace=trace,
    )
    y = _host_assemble([r["yt"] for r in res.results])
    return y, res


def kernel(**inputs):
    y, _ = _run(inputs)
    return y


def _make_sharded_fn(nc):
    """Mimic bass2jax.run_bass_via_pjrt's multi-core path, returning
    (sharded_fn, in_names, out_names, out_avals, mesh, n_params)."""
    import jax
    from jax.sharding import Mesh, PartitionSpec
    from jax.experimental.shard_map import shard_map
    from concourse import bass2jax, mybir
    import numpy as _np

    bass2jax.install_neuronx_cc_hook()
    partition_name = nc.partition_id_tensor.name if nc.partition_id_tensor else None
    in_names, out_names, out_avals = [], [], []
    for alloc in nc.m.functions[0].allocations:
        if not isinstance(alloc, mybir.MemoryLocationSet):
            continue
        name = alloc.memorylocations[0].name
        if alloc.kind == "ExternalInput":
            if name != partition_name:
                in_names.append(name)
        elif alloc.kind == "ExternalOutput":
            out_names.append(name)
            out_avals.append(jax.core.ShapedArray(tuple(alloc.tensor_shape), mybir.dt.np(alloc.dtype)))
    n_params = len(in_names)
    all_in_names = list(in_names) + list(out_names)
    if partition_name is not None:
        all_in_names.append(partition_name)
    donate = tuple(range(n_params, n_params + len(out_avals)))

    def _body(*args):
        operands = list(args)
        if partition_name is not None:
            operands.append(bass2jax.partition_id_tensor())
        outs = bass2jax._bass_exec_p.bind(
            *operands,
            out_avals=tuple(out_avals),
            in_names=tuple(all_in_names),
            out_names=tuple(out_names),
            lowering_input_output_aliases=(),
            sim_require_finite=True,
            sim_require_nnan=True,
            nc=nc,
        )
        return tuple(outs)

    devices = jax.devices()[:N_CORES]
    mesh = Mesh(_np.asarray(devices), ("core",))
    in_specs = (PartitionSpec("core"),) * (n_params + len(out_avals))
    out_specs = (PartitionSpec("core"),) * len(out_names)
    sharded = jax.jit(
        shard_map(_body, mesh=mesh, in_specs=in_specs, out_specs=out_specs, check_rep=False),
        donate_argnums=donate,
        keep_unused=True,
    )
    return sharded, in_names, out_names, out_avals, mesh, n_params


def bench(inputs, iters=30, warmup=3, **build_kw):
    """Time repeated on-device executions (inputs device-resident).

    Returns (per_iter_ns, y) where per_iter_ns is the best average over
    the timed iterations.
    """
    import time as _time
    import jax
    from jax.sharding import NamedSharding, PartitionSpec

    nc = _get_program(**build_kw)
    x1t, x2rep, wt = _host_prep(inputs)
    per_core = {
        "x1t": [np.ascontiguousarray(x1t[:, :, slice(c * NPC, (c + 1) * NPC)]) for c in range(N_CORES)],
        "x2r": [np.ascontiguousarray(x2rep[:, :, slice(c * NPC, (c + 1) * NPC)]) for c in range(N_CORES)],
        "wt": [wt] * N_CORES,
    }
    sharded, in_names, out_names, out_avals, mesh, n_params = _make_sharded_fn(nc)
    sh = NamedSharding(mesh, PartitionSpec("core"))
    dev_in = [
        jax.device_put(np.concatenate(per_core[name], axis=0), sh) for name in in_names
    ]

    def zeros():
        return [
            jax.device_put(np.zeros((N_CORES * a.shape[0], *a.shape[1:]), a.dtype), sh)
            for a in out_avals
        ]

    outs = None
    for _ in range(warmup):
        outs = sharded(*dev_in, *zeros())
        jax.block_until_ready(outs)

    # pre-stage zero buffers outside the timed region
    zs = [zeros() for _ in range(iters)]
    jax.block_until_ready(zs)
    t0 = _time.perf_counter()
    res = [sharded(*dev_in, *z) for z in zs]
    jax.block_until_ready(res)
    dt = (_time.perf_counter() - t0) / iters

    y_cat = np.asarray(res[-1][out_names.index("yt")])
    yt_cores = [y_cat[c * 128:(c + 1) * 128] for c in range(N_CORES)]
    y = _host_assemble(yt_cores)
    return dt * 1e9, y


if __name__ == "__main__":
    print("passes:", N_PASSES, "used_ij:", len(USED_IJ))


# revision 18
# speedup vs baseline: 1.3104x; 1.0341x over previous
"""Trainium2 Bass kernel for a dense Clebsch-Gordan tensor product + per-irrep Linear.

Reference computation (e3nn-style):
  x1: [N, 1152] = 128x0e + 128x1o + 128x2e   (mul=128, l=0,1,2)
  x2: [N, 9]    = 1x0e + 1x1o + 1x2e         (spherical harmonics)
  y[n, (g, v*d3+k)] = sum_{paths p in g} sum_{u,i,j} w_g[slot_p*128+u, v]/sqrt(mul_g)
                       * cg_p[i,j,k] * x1_{l1}[n,u,i] * x2_{l2}[n,j]

Strategy (data-parallel over N across 8 cores; per core n=1024):
  - host precomputes one scaled weight matrix W~[u,v] per CG nonzero (244, fp16),
    plus fp16 relayouts x1T[u; i_glob, n] and x2rep[128; j_glob, n]
  - DVE builds outer-product columns OP_{ij}[u; n] = x1T[:, i, :] * x2rep[:, j, :]
  - one fp16 matmul per CG nonzero accumulates y_psum[(g,k)][v; n] += W~.T @ OP_{ij}
  - PSUM drains to fp16 and DMAs out TRANSPOSED as y_t[v; (g,k), n]; the host
    reassembles y[n, 5120] fp32 (no on-device transposes at all)
"""

import sys
from math import factorial

import numpy as np

if "/opt/trn_rl_repo" not in sys.path:
    sys.path.insert(0, "/opt/trn_rl_repo")

MUL = 128
N_TOTAL = 8192
N_CORES = 8
NPC = N_TOTAL // N_CORES  # 1024 rows per core
IN1 = [(0, 1), (1, -1), (2, 1)]
IN2 = [(0, 1), (1, -1), (2, 1)]

# --------------------------------------------------------------- CG tables ---


def _f(n):
    return float(factorial(n))


def _su2_cg(j1, j2, j3):
    C = np.zeros((2 * j1 + 1, 2 * j2 + 1, 2 * j3 + 1))
    if not (abs(j1 - j2) <= j3 <= j1 + j2):
        return C
    pref0 = np.sqrt((2 * j3 + 1) * _f(j1 + j2 - j3) * _f(j1 - j2 + j3) * _f(-j1 + j2 + j3) / _f(j1 + j2 + j3 + 1))
    for m1 in range(-j1, j1 + 1):
        for m2 in range(-j2, j2 + 1):
            m3 = m1 + m2
            if abs(m3) > j3:
                continue
            pref = pref0 * np.sqrt(_f(j3 + m3) * _f(j3 - m3) * _f(j1 - m1) * _f(j1 + m1) * _f(j2 - m2) * _f(j2 + m2))
            s = 0.0
            for k in range(j1 + j2 - j3 + 1):
                a = [k, j1 + j2 - j3 - k, j1 - m1 - k, j2 + m2 - k, j3 - j2 + m1 + k, j3 - j1 - m2 + k]
                if min(a) < 0:
                    continue
                s += (-1.0) ** k / np.prod([_f(t) for t in a])
            C[j1 + m1, j2 + m2, j3 + m3] = pref * s
    return C


def _q(l):
    q = np.zeros((2 * l + 1, 2 * l + 1), dtype=np.complex128)
    for m in range(-l, 0):
        q[l + m, l + abs(m)] = 1 / np.sqrt(2)
        q[l + m, l - abs(m)] = -1j / np.sqrt(2)
    q[l, l] = 1.0
    for m in range(1, l + 1):
        q[l + m, l + abs(m)] = (-1) ** m / np.sqrt(2)
        q[l + m, l - abs(m)] = 1j * (-1) ** m / np.sqrt(2)
    return (-1j) ** l * q


def _real_cg(l1, l2, l3):
    C = _su2_cg(l1, l2, l3).astype(np.complex128)
    C = np.einsum("ij,kl,mn,ikn->jlm", _q(l1), _q(l2), np.conj(_q(l3).T), C)
    return np.real(C)


PATHS = []
for (l1, p1) in IN1:
    for (l2, p2) in IN2:
        for l3 in range(abs(l1 - l2), l1 + l2 + 1):
            PATHS.append((l1, p1, l2, p2, l3, p1 * p2))
CG = {(l1, l2, l3): _real_cg(l1, l2, l3).astype(np.float32) for (l1, _, l2, _, l3, _) in PATHS}
GROUPS = sorted({(l3, p3) for (_, _, _, _, l3, p3) in PATHS})


def _gname(l, p):
    return "w%d%s" % (l, "e" if p == 1 else "o")


L1_OFF = {0: 0, 1: 1, 2: 4}   # i_glob = L1_OFF[l1] + i
X1_OFF = {0: 0, 1: 128, 2: 512}  # x1 flat col offset of l1 block
L2_OFF = {0: 0, 1: 1, 2: 4}   # j_glob = L2_OFF[l2] + j

MULS = {g: 0 for g in GROUPS}
for (_, _, _, _, l3, p3) in PATHS:
    MULS[(l3, p3)] += MUL

GOFF = {}
_off = 0
for g in GROUPS:
    GOFF[g] = _off
    _off += MUL * (2 * g[0] + 1)
assert _off == 5120


def _build_pass_list():
    """gk_passes: per (g,k) in output order, list of (ij, path_idx, coef)."""
    gk_passes = []
    for g in GROUPS:
        d3 = 2 * g[0] + 1
        for k in range(d3):
            contribs = []
            for pi, (l1, p1, l2, p2, l3, p3) in enumerate(PATHS):
                if (l3, p3) != g:
                    continue
                C = CG[(l1, l2, l3)]
                for i in range(2 * l1 + 1):
                    for j in range(2 * l2 + 1):
                        c = float(C[i, j, k])
                        if abs(c) < 1e-8:
                            continue
                        contribs.append(((L1_OFF[l1] + i, L2_OFF[l2] + j), pi, c))
            assert contribs
            gk_passes.append((g, k, contribs))
    used = []
    seen = set()
    for (_, _, contribs) in gk_passes:
        for (ij, _, _) in contribs:
            if ij not in seen:
                seen.add(ij)
                used.append(ij)
    return gk_passes, used


GK_PASSES, USED_IJ = _build_pass_list()


def _reorder_passes(gk_passes):
    """Schedule order for the 40 (g,k) outputs.

    Put the (1o,*) outputs first: their contributions only touch OP tiles
    ig 0..3, so the PE can start before the DVE has built all 9 outer-product
    tiles of chunk 0.  Put a minimal-work gk last so the final
    drain+DMA tail after the last matmul is as short as possible.
    """
    def igs_needed(entry):
        return {ij[0] for (ij, _, _) in entry[2]}

    first = [e for e in gk_passes if e[0] == (1, -1)]
    rest = [e for e in gk_passes if e[0] != (1, -1)]
    # last: fewest contribs
    last = min(rest, key=lambda e: len(e[2]))
    rest.remove(last)
    return first + rest + [last]


GK_PASSES = _reorder_passes(GK_PASSES)

# slab index of each (g, k) in the device output y_t (scheduling order)
SLAB_OF = {(g, k): idx for idx, (g, k, _) in enumerate(GK_PASSES)}
N_GK = len(GK_PASSES)  # 40

# slot groups: first group = 3x(1o), then 4s, last = the small single gk
SLOT_GROUPS = [[0, 1, 2]] + [[3 + 4 * t + s for s in range(4)] for t in range(9)] + [[39]]
assert sorted(sum(SLOT_GROUPS, [])) == list(range(N_GK))

N_PASSES = sum(len(c) for (_, _, c) in GK_PASSES)  # 244

# Many passes share the same (path, coefficient): only 89 of the 244 scaled
# weight matrices are distinct.  The device weight stack holds one slab per
# unique (path, coef), ordered by first use in the schedule; PASS_POS maps
# each (gk-order, contrib) pass to its slab.
_uniq = {}
PASS_POS = {}
WT_SLABS = []   # [(path_idx, coef)] in first-use order
for gidx, (_, _, contribs) in enumerate(GK_PASSES):
    for m, (_, pi, c) in enumerate(contribs):
        key = (pi, round(c, 7))
        if key not in _uniq:
            _uniq[key] = len(WT_SLABS)
            WT_SLABS.append((pi, c))
        PASS_POS[(gidx, m)] = _uniq[key]
N_SLABS = len(WT_SLABS)  # 89
# last slab index needed by each slot group (for JIT weight DMA split)
GRP_MAX_SLAB = []
for grp in SLOT_GROUPS:
    mx = 0
    for gidx in grp:
        for m in range(len(GK_PASSES[gidx][2])):
            mx = max(mx, PASS_POS[(gidx, m)])
    GRP_MAX_SLAB.append(mx)


def _host_prep(inputs):
    """Host-side layout prep: x1T, x2rep (fp16) and the scaled weight stack."""
    x1 = np.asarray(inputs["x1"], np.float32)
    x2 = np.asarray(inputs["x2"], np.float32)
    n = x1.shape[0]

    x1t = np.empty((128, 9, n), np.float16)
    for (l1, _) in IN1:
        d1 = 2 * l1 + 1
        blk = x1[:, X1_OFF[l1]:X1_OFF[l1] + MUL * d1].reshape(n, MUL, d1)
        for i in range(d1):
            x1t[:, L1_OFF[l1] + i, :] = blk[:, :, i].astype(np.float16).T

    x2t = x2.astype(np.float16).T  # [9, n]
    x2rep = np.ascontiguousarray(np.broadcast_to(x2t[None, :, :], (128, 9, n)))

    # per-path weight slices (with e3nn path normalization)
    W = {g: np.asarray(inputs[_gname(*g)], np.float32) for g in GROUPS}
    slot = {g: 0 for g in GROUPS}
    path_w = []
    for (l1, p1, l2, p2, l3, p3) in PATHS:
        g = (l3, p3)
        s = slot[g]
        slot[g] += 1
        path_w.append(W[g][s * MUL:(s + 1) * MUL, :] / np.sqrt(np.float32(MULS[g])))

    wt = np.empty((MUL, N_SLABS, MUL), np.float16)   # [u, slab, v]
    for si, (pi, c) in enumerate(WT_SLABS):
        wt[:, si, :] = (path_w[pi] * np.float32(c)).astype(np.float16)
    return x1t, x2rep, wt


def _host_assemble(yt_cores):
    """yt per core: [128(v), N_GK, npc] -> y [N, 5120] fp32."""
    n_cores = len(yt_cores)
    npc = yt_cores[0].shape[2]
    y = np.empty((n_cores * npc, 5120), np.float32)
    for ci, yt in enumerate(yt_cores):
        rows = slice(ci * npc, (ci + 1) * npc)
        for g in GROUPS:
            d3 = 2 * g[0] + 1
            slabs = [SLAB_OF[(g, k)] for k in range(d3)]
            blk = yt[:, slabs, :]                   # [v, k, n]
            y[rows, GOFF[g]:GOFF[g] + MUL * d3] = (
                blk.transpose(2, 0, 1).reshape(npc, MUL * d3)
            )
    return y


# --------------------------------------------------------------- bass build ---

_CACHE = {}


def _build(n_per_core=NPC, chunks="128,256,256,256,128", op_bufs=18,
           yacc_bufs=4, ystg_bufs=6, act_share=0):
    """Build the per-core Bass/Tile program (v3: transposed output).

    Layouts:
      x1t  [u; i_glob(9), n]   fp16 (host-prepped); per-chunk SBUF tiles
      x2r  [128; j_glob(9), n] fp16 (host-prepped broadcast); per-chunk tiles
      wt   [u; pass(244), v]   fp16 scaled weights in scheduling order,
                               split into one SBUF tile per slot group
      y_t  [128(v), N_GK(40), n] fp16 output (host assembles y[n, 5120] fp32)

    Per n-chunk: DVE builds outer-product tiles OP_(i,*)[u; j, n]; one
    PSUM-accumulating fp16 matmul per CG nonzero into per-slot-group
    accumulator tiles [v; slot, n]; Act drains slots to fp16 staging; DMA
    writes the staging tile straight into y_t's (gk, chunk) slab.
    No PE transposes.
    """
    import concourse.bass as bass
    import concourse.mybir as mybir
    import concourse.tile as tile

    dt = mybir.dt
    NCROWS = n_per_core
    if isinstance(chunks, str):
        chunks = [int(x) for x in chunks.split(",")]
    assert sum(chunks) == NCROWS
    CH = len(chunks)
    coff = [sum(chunks[:c]) for c in range(CH)]

    flat = GK_PASSES
    n_gk = len(flat)
    groups = SLOT_GROUPS
    # every slot group must be a contiguous run of gk indices
    for grp in groups:
        assert grp == list(range(grp[0], grp[0] + len(grp)))

    # pass-index ranges per slot group (wt is gk-major so these are contiguous)
    grp_pass_range = []
    pc = 0
    for grp in groups:
        p0 = pc
        for gidx in grp:
            pc += len(flat[gidx][2])
        grp_pass_range.append((p0, pc))
    assert pc == N_PASSES

    nc = bass.Bass()

    x1t_d = nc.dram_tensor("x1t", [128, 9, NCROWS], dt.float16, kind="ExternalInput")
    x2r_d = nc.dram_tensor("x2r", [128, 9, NCROWS], dt.float16, kind="ExternalInput")
    wt_d = nc.dram_tensor("wt", [MUL, N_SLABS, MUL], dt.float16, kind="ExternalInput")
    yt_d = nc.dram_tensor("yt", [128, n_gk, NCROWS], dt.float16, kind="ExternalOutput")

    with tile.TileContext(nc) as tc:
        with (
            tc.tile_pool(name="const", bufs=1) as constp,
            tc.tile_pool(name="op", bufs=op_bufs) as opp,
            tc.tile_pool(name="ystg", bufs=ystg_bufs) as ystgp,
            tc.tile_pool(name="yacc", bufs=yacc_bufs, space="PSUM") as yaccp,
        ):
            x1c = [constp.tile([128, 9, chunks[c]], dt.float16, name=f"x1c{c}")
                   for c in range(CH)]
            x2c = [constp.tile([128, 9, chunks[c]], dt.float16, name=f"x2c{c}")
                   for c in range(CH)]
            wt_sb = constp.tile([128, N_SLABS, 128], dt.float16, name="wt_stack")

            # chunk-0 x + the weight slabs go just-in-time on the SP queue;
            # the later chunks' x tiles go on the Act queue (idle early, its
            # output DMAs only start once compute is underway).  Weight DMA
            # is split at the slab boundaries the first chunk's slot groups
            # need, in first-use order.
            def _load_x(c):
                n0 = coff[c]
                nc.sync.dma_start(x1c[c][:], x1t_d[:, :, n0:n0 + chunks[c]])
                nc.sync.dma_start(x2c[c][:], x2r_d[:, :, n0:n0 + chunks[c]])

            wt_cuts = sorted({GRP_MAX_SLAB[t] + 1 for t in range(len(groups))}
                             | {N_SLABS})
            wt_pieces = [(a, b) for a, b in zip([0] + wt_cuts, wt_cuts) if b > a]

            def _load_wt_piece(i):
                if i < len(wt_pieces):
                    a, b = wt_pieces[i]
                    nc.sync.dma_start(wt_sb[:, a:b, :], wt_d[:, a:b, :])

            _load_x(0)
            for i in (0, 1, 2):
                _load_wt_piece(i)
            for c in range(1, CH):
                _load_x(c)
                for i in range(3 * c, 3 * c + 3):
                    _load_wt_piece(i)
            for i in range(3 * CH, len(wt_pieces)):
                _load_wt_piece(i)

            for c in range(CH):
                n0 = coff[c]
                ncols = chunks[c]

                op_tiles = {}
                for ig in range(9):
                    t_ = opp.tile([128, 9, ncols], dt.float16, tag="op",
                                  name=f"op_c{c}_i{ig}")
                    a_b = x1c[c][:, ig, :].unsqueeze(1)
                    a_b = a_b.broadcast_to([128, 9, ncols])
                    nc.vector.tensor_mul(t_[:], a_b, x2c[c][:])
                    for jg in range(9):
                        op_tiles[(ig, jg)] = t_[:, jg, :]

                for t_idx, grp in enumerate(groups):
                    gsz = len(grp)
                    acc = yaccp.tile([128, gsz, ncols], dt.float32,
                                     tag="yacc", name=f"acc_c{c}_t{t_idx}")
                    ystg = ystgp.tile([128, gsz, ncols], dt.float16,
                                      tag="ystg", name=f"ystg_c{c}_t{t_idx}")
                    for s, gidx in enumerate(grp):
                        (g, k, contribs) = flat[gidx]
                        nmm = len(contribs)
                        for m, (ij, _, _) in enumerate(contribs):
                            nc.tensor.matmul(
                                acc[:, s, :],
                                wt_sb[:, PASS_POS[(gidx, m)], :],
                                op_tiles[ij],
                                start=(m == 0),
                                stop=(m == nmm - 1),
                            )
                    # alternate the PSUM->fp16 drains between Act and DVE so
                    # neither engine's drain chain lags the PE's PSUM ring
                    # (gpsimd cannot access PSUM)
                    if (c * len(groups) + t_idx) % 2 == 0:
                        nc.scalar.copy(ystg[:], acc[:])
                    else:
                        nc.vector.tensor_copy(ystg[:], acc[:])
                    s0 = grp[0]
                    nc.scalar.dma_start(
                        yt_d[:, s0:s0 + gsz, n0:n0 + ncols], ystg[:]
                    )

    return nc


def _hoist_waits(nc, max_waits=1):
    """Walrus in this toolchain rejects instructions with more than one
    sync-wait command; hoist extras onto single-wait NOP/Drain carriers that
    precede the instruction on the same engine."""
    import concourse.mybir as mybir

    n_hoisted = 0
    for bb in nc.main_func.blocks:
        new_list = []
        for ins in bb.instructions:
            si = ins.sync_info
            if si is not None and si.on_wait and len(si.on_wait) > max_waits:
                waits = list(si.on_wait)
                keep, hoist = waits[:max_waits], waits[max_waits:]
                for w in hoist:
                    carrier = mybir.InstDrain(
                        name=nc.get_next_instruction_name(),
                        ins=[], outs=[], bass_is_fusable=False)
                    carrier.engine = ins.engine
                    carrier.sync_info = mybir.SyncInfo(on_wait=[w], on_update=[])
                    new_list.append(carrier)
                    n_hoisted += 1
                ins.sync_info = mybir.SyncInfo(on_wait=keep, on_update=list(si.on_update))
            new_list.append(ins)
        bb.instructions[:] = new_list
    return n_hoisted


def _get_program(**kw):
    key = tuple(sorted(kw.items()))
    if key not in _CACHE:
        nc = _build(**kw)
        _hoist_waits(nc)
        _CACHE[key] = nc
    return _CACHE[key]


def _run(inputs, trace=False, **build_kw):
    from concourse import bass_utils

    nc = _get_program(**build_kw)
    x1t, x2rep, wt = _host_prep(inputs)

    in_maps = []
    for core in range(N_CORES):
        sl = slice(core * NPC, (core + 1) * NPC)
        in_maps.append({
            "x1t": np.ascontiguousarray(x1t[:, :, sl]),
            "x2r": np.ascontiguousarray(x2rep[:, :, sl]),
            "wt": wt,
        })

    res = bass_utils.run_bass_kernel_spmd(
        nc, in_maps, core_ids=list(range(N_CORES)), trace=trace,
    )
    y = _host_assemble([r["yt"] for r in res.results])
    return y, res


def kernel(**inputs):
    y, _ = _run(inputs)
    return y


def _make_sharded_fn(nc):
    """Mimic bass2jax.run_bass_via_pjrt's multi-core path, returning
    (sharded_fn, in_names, out_names, out_avals, mesh, n_params)."""
    import jax
    from jax.sharding import Mesh, PartitionSpec
    from jax.experimental.shard_map import shard_map
    from concourse import bass2jax, mybir
    import numpy as _np

    bass2jax.install_neuronx_cc_hook()
    partition_name = nc.partition_id_tensor.name if nc.partition_id_tensor else None
    in_names, out_names, out_avals = [], [], []
    for alloc in nc.m.functions[0].allocations:
        if not isinstance(alloc, mybir.MemoryLocationSet):
            continue
        name = alloc.memorylocations[0].name
        if alloc.kind == "ExternalInput":
            if name != partition_name:
                in_names.append(name)
        elif alloc.kind == "ExternalOutput":
            out_names.append(name)
            out_avals.append(jax.core.ShapedArray(tuple(alloc.tensor_shape), mybir.dt.np(alloc.dtype)))
    n_params = len(in_names)
    all_in_names = list(in_names) + list(out_names)
    if partition_name is not None:
        all_in_names.append(partition_name)
    donate = tuple(range(n_params, n_params + len(out_avals)))

    def _body(*args):
        operands = list(args)
        if partition_name is not None:
            operands.append(bass2jax.partition_id_tensor())
        outs = bass2jax._bass_exec_p.bind(
            *operands,
            out_avals=tuple(out_avals),
            in_names=tuple(all_in_names),
            out_names=tuple(out_names),
            lowering_input_output_aliases=(),
            sim_require_finite=True,
            sim_require_nnan=True,
            nc=nc,
        )
        return tuple(outs)

    devices = jax.devices()[:N_CORES]
    mesh = Mesh(_np.asarray(devices), ("core",))
    in_specs = (PartitionSpec("core"),) * (n_params + len(out_avals))
    out_specs = (PartitionSpec("core"),) * len(out_names)
    sharded = jax.jit(
        shard_map(_body, mesh=mesh, in_specs=in_specs, out_specs=out_specs, check_rep=False),
        donate_argnums=donate,
        keep_unused=True,
    )
    return sharded, in_names, out_names, out_avals, mesh, n_params


def bench(inputs, iters=30, warmup=3, **build_kw):
    """Time repeated on-device executions (inputs device-resident).

    Returns (per_iter_ns, y) where per_iter_ns is the best average over
    the timed iterations.
    """
    import time as _time
    import jax
    from jax.sharding import NamedSharding, PartitionSpec

    nc = _get_program(**build_kw)
    x1t, x2rep, wt = _host_prep(inputs)
    per_core = {
        "x1t": [np.ascontiguousarray(x1t[:, :, slice(c * NPC, (c + 1) * NPC)]) for c in range(N_CORES)],
        "x2r": [np.ascontiguousarray(x2rep[:, :, slice(c * NPC, (c + 1) * NPC)]) for c in range(N_CORES)],
        "wt": [wt] * N_CORES,
    }
    sharded, in_names, out_names, out_avals, mesh, n_params = _make_sharded_fn(nc)
    sh = NamedSharding(mesh, PartitionSpec("core"))
    dev_in = [
        jax.device_put(np.concatenate(per_core[name], axis=0), sh) for name in in_names
    ]

    def zeros():
        return [
            jax.device_put(np.zeros((N_CORES * a.shape[0], *a.shape[1:]), a.dtype), sh)
            for a in out_avals
        ]

    outs = None
    for _ in range(warmup):
        outs = sharded(*dev_in, *zeros())
        jax.block_until_ready(outs)

    # pre-stage zero buffers outside the timed region
    zs = [zeros() for _ in range(iters)]
    jax.block_until_ready(zs)
    t0 = _time.perf_counter()
    res = [sharded(*dev_in, *z) for z in zs]
    jax.block_until_ready(res)
    dt = (_time.perf_counter() - t0) / iters

    y_cat = np.asarray(res[-1][out_names.index("yt")])
    yt_cores = [y_cat[c * 128:(c + 1) * 128] for c in range(N_CORES)]
    y = _host_assemble(yt_cores)
    return dt * 1e9, y


if __name__ == "__main__":
    print("passes:", N_PASSES, "used_ij:", len(USED_IJ))
